# revision 18
# baseline (speedup 1.0000x reference)
"""Trainium2 Bass kernel for nn_Decoder: out = (x - b_pre) @ W^T.

Shapes (hardcoded): x [8192, 32768] f32, W [768, 32768] f32, b_pre
[32768] f32 -> out [8192, 768] f32.

Sharding: data-parallel over the 8192 token rows across 8 NeuronCores
(1024 rows each), W replicated. The TensorE contracts over the
partition axis, so the host pre-transposes both operands to put the
contraction dim d on partitions; b_pre is folded into x on the host
(bitwise no-op for the reference's b_pre == 0).

Numerics/speed (measured on this machine's trn2 cores):
  - The PE streams exactly 1 output column per cycle at ~2.4 GHz for
    every non-fp32 input dtype, including fp8-e4m3 in DoubleRow perf
    mode (measured 109 ns per 256-col matmul for fp16 AND fp8-DR, 216
    ns per 512-col; DoubleRow gives NO MAC-rate advantage here - its
    value is halved cold-start bytes, see the last paragraph).
  - float16 inputs beat the old float32r version on the other PE pipe:
    each matmul's LDWEIGHTS takes ~116 ns in fp16 vs ~190 ns in f32r,
    and f32r was LDWEIGHTS-pipe-bound (~190 ns/matmul). fp16 is
    stream-bound instead: (216+109) ns per 768-col pair.
  - fp16 input rounding gives 3.13e-4 scale-relative error vs the 2e-2
    gate (PSUM accumulation stays fp32).

Structure: d is processed in supers of 16 chunks x 128 rows. Each chunk
DMAs xT [128, 1024] (2 KB/partition) on the SP queue and wT [128, 768]
(1.5 KB/partition) on the ACT queue - both tensors touch HBM exactly
once (~105 MB/core, far under the DMA roof). Per 128-token block a
[128, 768] PSUM tile accumulates the super's 16 chunks (2 matmuls per
chunk: 512 + 256 cols, one accumulation chain per PSUM bank), then the
DVE adds it into a [1024, 768] f32 SBUF-resident C that DMAs out at
the end. 40-deep chunk pools (2.5 supers of prefetch) keep DMA waits
off the PE: matmul start-deltas sit at the light-load floor (109/216
ns p50) and total LDWEIGHTS wait is ~28 us.

Measured HW exec: 688034 ns with the fp8 warm-start below (692817 ns
pure-fp16; prior float32r baseline: 773-826 us; stream-time floor for
this structure is ~666 us + ~11 us framework preamble + ~12 us
epilogue). Deeper buffering (44), psum bufs 3 + 32-chunk supers (828
us), 2-DMA-per-super (698 us), fine-first-super + coarse-rest (699
us), gpsimd cold-start DMAs (708 us), and ramped super sizes (695 us)
all measured neutral or worse.

The fp8-e4m3 DoubleRow path (KERNEL_FP8_CHUNKS=N runs the first N
256-row d-chunks in fp8 with W pre-scaled by 128 - it sits in e4m3's
subnormal range otherwise - descaled in a fused DVE multiply-add;
validated bit-exact vs a quantized host reference in CoreSim) runs at
the SAME PE rate as fp16, but halves the cold-start HBM bytes: with
N=8 (first 2048 d-rows) the opening super's data lands ~2x sooner, so
the warm-up stalls drop from ~14 us to ~6 us while the fp16 chunk
prefetch fills behind it. Its psum chains borrow psum16 [128,768]
slots (using partitions 0:64) so PS16_BUFS=3 still fits the 8 PSUM
banks. Default N=8: measured 688034 ns at rel err 1.001e-2
(deterministic, 2x under the 2e-2 gate; set KERNEL_FP8_CHUNKS=0 for
the pure-fp16 693 us / 3.13e-4 configuration).
"""

import os
import sys

if "/opt/trn_rl_repo" not in sys.path:
    sys.path.insert(0, "/opt/trn_rl_repo")

import numpy as np

N_TOK = 8192
D_IN = 32768
D_OUT = 768
N_CORES = 8
N_SHARD = N_TOK // N_CORES          # 1024 token rows per core
P = 128

C8 = int(os.environ.get("KERNEL_FP8_CHUNKS", "8"))
D8 = C8 * 256
D16 = D_IN - D8
C16 = D16 // P                      # fp16 d-chunks of 128 rows

X8_BUFS = int(os.environ.get("KERNEL_X8_BUFS", "10"))
W8_BUFS = int(os.environ.get("KERNEL_W8_BUFS", "10"))
X16_BUFS = int(os.environ.get("KERNEL_X16_BUFS", "40"))
W16_BUFS = int(os.environ.get("KERNEL_W16_BUFS", "40"))
PS8_BUFS = int(os.environ.get("KERNEL_PS8_BUFS", "2"))
# psum16 triple-buffering measured ~5 us faster; the fp8 chains borrow
# psum16 slots (partitions 0:64) so 3 bufs fit the 8 banks either way.
PS16_BUFS = int(os.environ.get("KERNEL_PS16_BUFS", "3"))
SUP8 = 8                            # fp8 chunks per super
SUP16 = int(os.environ.get("KERNEL_SUP16", "16"))  # fp16 chunks per super

LAST_RESULTS = None  # BassKernelResults of the most recent kernel() call


def _build_bass(c8):
    import concourse.mybir as mybir
    import concourse.tile as tile
    from concourse import bacc

    fp8 = mybir.dt.float8e4
    fp16 = mybir.dt.float16
    f32 = mybir.dt.float32
    c16 = (D_IN - c8 * 256) // P
    NCH = N_SHARD // P              # 8 output row-chunks of 128 tokens

    nc = bacc.Bacc(None, target_bir_lowering=False)
    if c8:
        xP8 = nc.dram_tensor("xP8", [c8, P, 2, N_SHARD], fp8,
                             kind="ExternalInput")
        wP8 = nc.dram_tensor("wP8", [c8, P, 2, D_OUT], fp8,
                             kind="ExternalInput")
    if c16:
        xT16 = nc.dram_tensor("xT16", [c16 * P, N_SHARD], fp16,
                              kind="ExternalInput")
        wT16 = nc.dram_tensor("wT16", [c16 * P, D_OUT], fp16,
                              kind="ExternalInput")
    out = nc.dram_tensor("out", [N_SHARD, D_OUT], f32,
                         kind="ExternalOutput")

    from contextlib import ExitStack

    with tile.TileContext(nc) as tc, ExitStack() as es:
        if c8:
            x8pool = es.enter_context(tc.tile_pool(name="x8", bufs=X8_BUFS))
            w8pool = es.enter_context(tc.tile_pool(name="w8", bufs=W8_BUFS))
        if c16:
            x16pool = es.enter_context(
                tc.tile_pool(name="x16", bufs=X16_BUFS))
            w16pool = es.enter_context(
                tc.tile_pool(name="w16", bufs=W16_BUFS))
            ppool16 = es.enter_context(
                tc.tile_pool(name="psum16", bufs=PS16_BUFS, space="PSUM"))
        cpool = es.enter_context(tc.tile_pool(name="c", bufs=1))
        if True:
            cts = [
                cpool.tile([P, D_OUT], f32, name=f"c{i}") for i in range(NCH)
            ]
            first = True

            # fp8 DoubleRow supers (chunks of 256 d-rows)
            for s0 in range(0, c8, SUP8):
                js = list(range(s0, min(s0 + SUP8, c8)))
                xts, wts = [], []
                for j in js:
                    xt = x8pool.tile([P, 2, N_SHARD], fp8, name="xt8")
                    wt = w8pool.tile([P, 2, D_OUT], fp8, name="wt8")
                    nc.sync.dma_start(xt[:], xP8[j])
                    nc.scalar.dma_start(wt[:], wP8[j])
                    xts.append(xt)
                    wts.append(wt)
                for nch in range(NCH):
                    for half in range(2):
                        # borrow a [128,768] psum16 slot; DoubleRow
                        # writes/reads only partitions 0:64 of it
                        ps128 = ppool16.tile([P, D_OUT], f32, name="ps16")
                        ps = ps128[0:64, :]
                        col = nch * P + half * 64
                        for ji, xt in enumerate(xts):
                            lhsT = xt[:, :, col:col + 64]
                            last = ji == len(xts) - 1
                            for oc in range(3):
                                # 2KB-bank chains: bank0 = oc0+oc1
                                # (one start, one stop), bank1 = oc2.
                                nc.tensor.matmul(
                                    ps[:, oc * 256:(oc + 1) * 256],
                                    lhsT,
                                    wts[ji][:, :, oc * 256:(oc + 1) * 256],
                                    start=(ji == 0 and oc in (0, 2)),
                                    stop=(last and oc in (1, 2)),
                                    perf_mode=mybir.MatmulPerfMode.DoubleRow,
                                )
                        dst = cts[nch][half * 64:(half + 1) * 64, :]
                        if first:
                            nc.vector.tensor_scalar_mul(dst, ps[:], 1.0 / 128.0)
                        else:
                            nc.vector.scalar_tensor_tensor(
                                dst, ps[:], 1.0 / 128.0, dst,
                                op0=mybir.AluOpType.mult,
                                op1=mybir.AluOpType.add,
                            )
                first = False

            # fp16 supers (chunks of 128 d-rows)
            for s0 in range(0, c16, SUP16):
                js = list(range(s0, min(s0 + SUP16, c16)))
                xts, wts = [], []
                for j in js:
                    xt = x16pool.tile([P, N_SHARD], fp16, name="xt16")
                    wt = w16pool.tile([P, D_OUT], fp16, name="wt16")
                    nc.sync.dma_start(xt[:], xT16[j * P:(j + 1) * P, :])
                    nc.scalar.dma_start(wt[:], wT16[j * P:(j + 1) * P, :])
                    xts.append(xt)
                    wts.append(wt)
                for nch in range(NCH):
                    ps = ppool16.tile([P, D_OUT], f32, name="ps16")
                    for ji, xt in enumerate(xts):
                        lhsT = xt[:, nch * P:(nch + 1) * P]
                        st = ji == 0
                        sp = ji == len(xts) - 1
                        nc.tensor.matmul(ps[:, 0:512], lhsT,
                                         wts[ji][:, 0:512], start=st, stop=sp)
                        nc.tensor.matmul(ps[:, 512:D_OUT], lhsT,
                                         wts[ji][:, 512:D_OUT],
                                         start=st, stop=sp)
                    last_tile = s0 + SUP16 >= c16 and nch == NCH - 1
                    if first:
                        nc.vector.tensor_copy(cts[nch][:], ps[:])
                    elif last_tile:
                        # split the final accumulate by column halves:
                        # the 0:512 psum chain stops one matmul early,
                        # so its add (and its half of the output DMA
                        # below) overlap the closing 512:768 matmul +
                        # add, shortening the serial tail after the
                        # last matmul from ~2.2 us to ~0.8 us.
                        nc.vector.tensor_add(cts[nch][:, 0:512],
                                             cts[nch][:, 0:512],
                                             ps[:, 0:512])
                        nc.vector.tensor_add(cts[nch][:, 512:D_OUT],
                                             cts[nch][:, 512:D_OUT],
                                             ps[:, 512:D_OUT])
                    else:
                        nc.vector.tensor_add(cts[nch][:], cts[nch][:], ps[:])
                first = False

            for nch in range(NCH):
                base = nch * P
                if nch == NCH - 1:
                    nc.sync.dma_start(out[base:base + P, 0:512],
                                      cts[nch][:, 0:512])
                    nc.scalar.dma_start(out[base:base + P, 512:D_OUT],
                                        cts[nch][:, 512:D_OUT])
                else:
                    nc.sync.dma_start(out[base:base + P, :], cts[nch][:])

    nc.compile()
    return nc


def _prep_inputs(x, W, c8):
    """Quantize + transpose + pack on the host. Returns per-core in_maps."""
    import ml_dtypes

    e4m3 = ml_dtypes.float8_e4m3
    d8 = c8 * 256
    in_maps = []
    if c8:
        w8 = np.ascontiguousarray((W[:, :d8].T * np.float32(128.0))
                                  .astype(e4m3)).reshape(c8, P, 2, D_OUT)
    if d8 < D_IN:
        w16 = np.ascontiguousarray(W[:, d8:].T.astype(np.float16))
    for c in range(N_CORES):
        xs = x[c * N_SHARD:(c + 1) * N_SHARD]
        m = {}
        if c8:
            m["xP8"] = np.ascontiguousarray(
                xs[:, :d8].T.astype(e4m3)).reshape(c8, P, 2, N_SHARD)
            m["wP8"] = w8
        if d8 < D_IN:
            m["xT16"] = np.ascontiguousarray(xs[:, d8:].T.astype(np.float16))
            m["wT16"] = w16
        in_maps.append(m)
    return in_maps


def _run_device(x, W, c8):
    global LAST_RESULTS
    from concourse.bass_utils import run_bass_kernel_spmd

    nc = _build_bass(c8)
    in_maps = _prep_inputs(x, W, c8)
    last_err = None
    for attempt in range(3):
        try:
            LAST_RESULTS = run_bass_kernel_spmd(
                nc, in_maps, core_ids=list(range(N_CORES)),
                tmpdir=os.environ.get("KERNEL_TRACE_DIR") or None,
            )
            break
        except Exception as e:  # transient device faults recover on retry
            last_err = e
            import time

            time.sleep(10)
    else:
        raise last_err
    return np.concatenate(
        [LAST_RESULTS.results[c]["out"] for c in range(N_CORES)], axis=0
    )


def kernel(x: np.ndarray, W: np.ndarray, b_pre: np.ndarray) -> np.ndarray:
    x = np.asarray(x, dtype=np.float32)
    W = np.asarray(W, dtype=np.float32)
    b_pre = np.asarray(b_pre, dtype=np.float32)

    # Fold the pre-bias on the host (exact no-op for b_pre == 0).
    if b_pre.any():
        x = x - b_pre[None, :]

    out = _run_device(x, W, C8)

    # Sampled sanity check (64 rows vs numpy fp64).
    idx = np.arange(0, N_TOK, N_TOK // 64)
    ref = x[idx].astype(np.float64) @ W.astype(np.float64).T
    err = np.abs(out[idx] - ref).max() / (np.abs(ref).max() + 1e-30)
    if not np.isfinite(err) or err > 2.6e-2:
        out = _run_device(x, W, 0)
    return out


# revision 19
# speedup vs baseline: 1.0002x; 1.0002x over previous
"""Trainium2 Bass kernel for nn_Decoder: out = (x - b_pre) @ W^T.

Shapes (hardcoded): x [8192, 32768] f32, W [768, 32768] f32, b_pre
[32768] f32 -> out [8192, 768] f32.

Sharding: data-parallel over the 8192 token rows across 8 NeuronCores
(1024 rows each), W replicated. The TensorE contracts over the
partition axis, so the host pre-transposes both operands to put the
contraction dim d on partitions; b_pre is folded into x on the host
(bitwise no-op for the reference's b_pre == 0).

Numerics/speed (measured on this machine's trn2 cores):
  - The PE streams exactly 1 output column per cycle at ~2.4 GHz for
    every non-fp32 input dtype, including fp8-e4m3 in DoubleRow perf
    mode (measured 109 ns per 256-col matmul for fp16 AND fp8-DR, 216
    ns per 512-col; DoubleRow gives NO MAC-rate advantage here - its
    value is halved cold-start bytes, see the last paragraph).
  - float16 inputs beat the old float32r version on the other PE pipe:
    each matmul's LDWEIGHTS takes ~116 ns in fp16 vs ~190 ns in f32r,
    and f32r was LDWEIGHTS-pipe-bound (~190 ns/matmul). fp16 is
    stream-bound instead: (216+109) ns per 768-col pair.
  - fp16 input rounding gives 3.13e-4 scale-relative error vs the 2e-2
    gate (PSUM accumulation stays fp32).

Structure: d is processed in supers of 16 chunks x 128 rows. Each chunk
DMAs xT [128, 1024] (2 KB/partition) on the SP queue and wT [128, 768]
(1.5 KB/partition) on the ACT queue - both tensors touch HBM exactly
once (~105 MB/core, far under the DMA roof). Per 128-token block a
[128, 768] PSUM tile accumulates the super's 16 chunks (2 matmuls per
chunk: 512 + 256 cols, one accumulation chain per PSUM bank), then the
DVE adds it into a [1024, 768] f32 SBUF-resident C that DMAs out at
the end. 40-deep chunk pools (2.5 supers of prefetch) keep DMA waits
off the PE: matmul start-deltas sit at the light-load floor (109/216
ns p50) and total LDWEIGHTS wait is ~28 us.

Measured HW exec: 688034 ns with the fp8 warm-start below (692817 ns
pure-fp16; prior float32r baseline: 773-826 us; stream-time floor for
this structure is ~666 us + ~11 us framework preamble + ~12 us
epilogue). Deeper buffering (44), psum bufs 3 + 32-chunk supers (828
us), 2-DMA-per-super (698 us), fine-first-super + coarse-rest (699
us), gpsimd cold-start DMAs (708 us), and ramped super sizes (695 us)
all measured neutral or worse.

The fp8-e4m3 DoubleRow path (KERNEL_FP8_CHUNKS=N runs the first N
256-row d-chunks in fp8 with W pre-scaled by 128 - it sits in e4m3's
subnormal range otherwise - descaled in a fused DVE multiply-add;
validated bit-exact vs a quantized host reference in CoreSim) runs at
the SAME PE rate as fp16, but halves the cold-start HBM bytes: with
N=8 (first 2048 d-rows) the opening super's data lands ~2x sooner, so
the warm-up stalls drop from ~14 us to ~6 us while the fp16 chunk
prefetch fills behind it. Its psum chains borrow psum16 [128,768]
slots (using partitions 0:64) so PS16_BUFS=3 still fits the 8 PSUM
banks. Default N=8: measured 688034 ns at rel err 1.001e-2
(deterministic, 2x under the 2e-2 gate; set KERNEL_FP8_CHUNKS=0 for
the pure-fp16 693 us / 3.13e-4 configuration).
"""

import os
import sys

if "/opt/trn_rl_repo" not in sys.path:
    sys.path.insert(0, "/opt/trn_rl_repo")

import numpy as np

N_TOK = 8192
D_IN = 32768
D_OUT = 768
N_CORES = 8
N_SHARD = N_TOK // N_CORES          # 1024 token rows per core
P = 128

C8 = int(os.environ.get("KERNEL_FP8_CHUNKS", "8"))
D8 = C8 * 256
D16 = D_IN - D8
C16 = D16 // P                      # fp16 d-chunks of 128 rows

X8_BUFS = int(os.environ.get("KERNEL_X8_BUFS", "10"))
W8_BUFS = int(os.environ.get("KERNEL_W8_BUFS", "10"))
X16_BUFS = int(os.environ.get("KERNEL_X16_BUFS", "40"))
W16_BUFS = int(os.environ.get("KERNEL_W16_BUFS", "40"))
PS8_BUFS = int(os.environ.get("KERNEL_PS8_BUFS", "2"))
# psum16 triple-buffering measured ~5 us faster; the fp8 chains borrow
# psum16 slots (partitions 0:64) so 3 bufs fit the 8 banks either way.
PS16_BUFS = int(os.environ.get("KERNEL_PS16_BUFS", "3"))
SUP8 = 8                            # fp8 chunks per super
SUP16 = int(os.environ.get("KERNEL_SUP16", "16"))  # fp16 chunks per super

LAST_RESULTS = None  # BassKernelResults of the most recent kernel() call


def _build_bass(c8):
    import concourse.mybir as mybir
    import concourse.tile as tile
    from concourse import bacc

    fp8 = mybir.dt.float8e4
    fp16 = mybir.dt.float16
    f32 = mybir.dt.float32
    c16 = (D_IN - c8 * 256) // P
    NCH = N_SHARD // P              # 8 output row-chunks of 128 tokens

    nc = bacc.Bacc(None, target_bir_lowering=False)
    if c8:
        xP8 = nc.dram_tensor("xP8", [c8, P, 2, N_SHARD], fp8,
                             kind="ExternalInput")
        wP8 = nc.dram_tensor("wP8", [c8, P, 2, D_OUT], fp8,
                             kind="ExternalInput")
    if c16:
        xT16 = nc.dram_tensor("xT16", [c16 * P, N_SHARD], fp16,
                              kind="ExternalInput")
        wT16 = nc.dram_tensor("wT16", [c16 * P, D_OUT], fp16,
                              kind="ExternalInput")
    out = nc.dram_tensor("out", [N_SHARD, D_OUT], f32,
                         kind="ExternalOutput")

    from contextlib import ExitStack

    with tile.TileContext(nc) as tc, ExitStack() as es:
        if c8:
            x8pool = es.enter_context(tc.tile_pool(name="x8", bufs=X8_BUFS))
            w8pool = es.enter_context(tc.tile_pool(name="w8", bufs=W8_BUFS))
        if c16:
            x16pool = es.enter_context(
                tc.tile_pool(name="x16", bufs=X16_BUFS))
            w16pool = es.enter_context(
                tc.tile_pool(name="w16", bufs=W16_BUFS))
            ppool16 = es.enter_context(
                tc.tile_pool(name="psum16", bufs=PS16_BUFS, space="PSUM"))
        cpool = es.enter_context(tc.tile_pool(name="c", bufs=1))
        if True:
            cts = [
                cpool.tile([P, D_OUT], f32, name=f"c{i}") for i in range(NCH)
            ]
            first = True

            # fp8 DoubleRow supers (chunks of 256 d-rows)
            for s0 in range(0, c8, SUP8):
                js = list(range(s0, min(s0 + SUP8, c8)))
                xts, wts = [], []
                for j in js:
                    xt = x8pool.tile([P, 2, N_SHARD], fp8, name="xt8")
                    wt = w8pool.tile([P, 2, D_OUT], fp8, name="wt8")
                    nc.sync.dma_start(xt[:], xP8[j])
                    nc.scalar.dma_start(wt[:], wP8[j])
                    xts.append(xt)
                    wts.append(wt)
                for nch in range(NCH):
                    for half in range(2):
                        # borrow a [128,768] psum16 slot; DoubleRow
                        # writes/reads only partitions 0:64 of it
                        ps128 = ppool16.tile([P, D_OUT], f32, name="ps16")
                        ps = ps128[0:64, :]
                        col = nch * P + half * 64
                        for ji, xt in enumerate(xts):
                            lhsT = xt[:, :, col:col + 64]
                            last = ji == len(xts) - 1
                            for oc in range(3):
                                # 2KB-bank chains: bank0 = oc0+oc1
                                # (one start, one stop), bank1 = oc2.
                                nc.tensor.matmul(
                                    ps[:, oc * 256:(oc + 1) * 256],
                                    lhsT,
                                    wts[ji][:, :, oc * 256:(oc + 1) * 256],
                                    start=(ji == 0 and oc in (0, 2)),
                                    stop=(last and oc in (1, 2)),
                                    perf_mode=mybir.MatmulPerfMode.DoubleRow,
                                )
                        dst = cts[nch][half * 64:(half + 1) * 64, :]
                        if first:
                            nc.vector.tensor_scalar_mul(dst, ps[:], 1.0 / 128.0)
                        else:
                            nc.vector.scalar_tensor_tensor(
                                dst, ps[:], 1.0 / 128.0, dst,
                                op0=mybir.AluOpType.mult,
                                op1=mybir.AluOpType.add,
                            )
                first = False

            # fp16 supers (chunks of 128 d-rows)
            for s0 in range(0, c16, SUP16):
                js = list(range(s0, min(s0 + SUP16, c16)))
                xts, wts = [], []
                for j in js:
                    xt = x16pool.tile([P, N_SHARD], fp16, name="xt16")
                    wt = w16pool.tile([P, D_OUT], fp16, name="wt16")
                    nc.sync.dma_start(xt[:], xT16[j * P:(j + 1) * P, :])
                    nc.scalar.dma_start(wt[:], wT16[j * P:(j + 1) * P, :])
                    xts.append(xt)
                    wts.append(wt)
                for nch in range(NCH):
                    ps = ppool16.tile([P, D_OUT], f32, name="ps16")
                    for ji, xt in enumerate(xts):
                        lhsT = xt[:, nch * P:(nch + 1) * P]
                        st = ji == 0
                        sp = ji == len(xts) - 1
                        nc.tensor.matmul(ps[:, 0:512], lhsT,
                                         wts[ji][:, 0:512], start=st, stop=sp)
                        nc.tensor.matmul(ps[:, 512:D_OUT], lhsT,
                                         wts[ji][:, 512:D_OUT],
                                         start=st, stop=sp)
                    if first:
                        nc.vector.tensor_copy(cts[nch][:], ps[:])
                    else:
                        nc.vector.tensor_add(cts[nch][:], cts[nch][:], ps[:])
                first = False

            for nch in range(NCH):
                nc.sync.dma_start(out[nch * P:(nch + 1) * P, :], cts[nch][:])

    nc.compile()
    return nc


def _prep_inputs(x, W, c8):
    """Quantize + transpose + pack on the host. Returns per-core in_maps."""
    import ml_dtypes

    e4m3 = ml_dtypes.float8_e4m3
    d8 = c8 * 256
    in_maps = []
    if c8:
        w8 = np.ascontiguousarray((W[:, :d8].T * np.float32(128.0))
                                  .astype(e4m3)).reshape(c8, P, 2, D_OUT)
    if d8 < D_IN:
        w16 = np.ascontiguousarray(W[:, d8:].T.astype(np.float16))
    for c in range(N_CORES):
        xs = x[c * N_SHARD:(c + 1) * N_SHARD]
        m = {}
        if c8:
            m["xP8"] = np.ascontiguousarray(
                xs[:, :d8].T.astype(e4m3)).reshape(c8, P, 2, N_SHARD)
            m["wP8"] = w8
        if d8 < D_IN:
            m["xT16"] = np.ascontiguousarray(xs[:, d8:].T.astype(np.float16))
            m["wT16"] = w16
        in_maps.append(m)
    return in_maps


def _run_device(x, W, c8):
    global LAST_RESULTS
    from concourse.bass_utils import run_bass_kernel_spmd

    nc = _build_bass(c8)
    in_maps = _prep_inputs(x, W, c8)
    last_err = None
    for attempt in range(3):
        try:
            LAST_RESULTS = run_bass_kernel_spmd(
                nc, in_maps, core_ids=list(range(N_CORES)),
                tmpdir=os.environ.get("KERNEL_TRACE_DIR") or None,
            )
            break
        except Exception as e:  # transient device faults recover on retry
            last_err = e
            import time

            time.sleep(10)
    else:
        raise last_err
    return np.concatenate(
        [LAST_RESULTS.results[c]["out"] for c in range(N_CORES)], axis=0
    )


def kernel(x: np.ndarray, W: np.ndarray, b_pre: np.ndarray) -> np.ndarray:
    x = np.asarray(x, dtype=np.float32)
    W = np.asarray(W, dtype=np.float32)
    b_pre = np.asarray(b_pre, dtype=np.float32)

    # Fold the pre-bias on the host (exact no-op for b_pre == 0).
    if b_pre.any():
        x = x - b_pre[None, :]

    out = _run_device(x, W, C8)

    # Sampled sanity check (64 rows vs numpy fp64).
    idx = np.arange(0, N_TOK, N_TOK // 64)
    ref = x[idx].astype(np.float64) @ W.astype(np.float64).T
    err = np.abs(out[idx] - ref).max() / (np.abs(ref).max() + 1e-30)
    if not np.isfinite(err) or err > 2.6e-2:
        out = _run_device(x, W, 0)
    return out
